# revision 6
# baseline (speedup 1.0000x reference)
"""Contourlet-SD (nlevs=[2,3,4], Pyr_mode=1, pkva ladder) Trainium2 kernel.

Strategy (validated in fp64 prototype, see repo notes):
  Pyramid level (n x n image, n in 1024/512/256):
    lowpass+decimate:  lp = D x D^T          (D = decimated circulant rows)
    highpass:          band = x - ifft2(fft2(x) * (1-Hm)).real, computed via
      half-spectrum cos/sin transforms restricted to the low-frequency corner
      (m+ ~ n/3 frequencies, zero-padded to MP[n]):
        corr = C^T(Gt*(CxC^T))C + C^T(Gt*(CxS^T))S
             + S^T(Gt*(SxC^T))C + S^T(Gt*(SxS^T))S ,  Gt = w G w / n^2.
      Gt is negated and the x-passthrough is realised as PE transposes
      accumulated into the same PSUM, producing band TRANSPOSED (DFB entry).
  DFB level ("P-form"): the split axis lives on SBUF partitions; s/d outputs
    are banded circulant matrix multiplies (ladder lifting folded into one
    matrix; modulation sign + quincunx roll folded into even/odd-column matrix
    variants; realised as <=4 stationary 128x128 band blocks per variant with
    parity-strided moving operands). Between levels the subband stack is
    transposed on the PE (f32r transpose mode).

  All matmuls run in float32r (tf32-like, ~1.6e-4 rel err, full PE rate at
  N>=256). Batch: 16 images over 8 cores = 2 images/core, SPMD.
"""
import sys
sys.path.insert(0, "/opt/trn_rl_repo")
import os
import numpy as np

P = 128
NS = (1024, 512, 256)
MP = {1024: 384, 512: 256, 256: 128}     # padded half-spectrum size
NLEV_DFB = {1024: 4, 512: 3, 256: 2}     # band size -> # DFB levels

_PKVA_HALF = np.array([0.6300, -0.1930, 0.0972, -0.0526, 0.0272, -0.0144])
PKVA = np.concatenate([_PKVA_HALF[::-1], _PKVA_HALF])


# ----------------------------------------------------------------------------
# host-side constants
# ----------------------------------------------------------------------------
def _phi(n):
    w = np.abs(2.0 * np.pi * np.fft.fftfreq(n))
    s = np.clip((w - np.pi / 3.0) / (np.pi / 3.0), 0.0, 1.0)
    beta = s ** 4 * (35.0 - 84.0 * s + 70.0 * s ** 2 - 20.0 * s ** 3)
    return np.cos(0.5 * np.pi * beta)


def _lp_D(n):
    """D (n/2, n): decimated rows of the circulant lowpass."""
    a = np.fft.ifft(_phi(n)).real
    i = np.arange(0, n, 2)[:, None]
    j = np.arange(n)[None, :]
    return a[(i - j) % n]


def _corr_CSG(n):
    """C,S (mp, n) cos/sin half-spectrum (zero-padded rows), Gneg (mp, mp)."""
    phi = _phi(n)
    m = int(np.floor(n / 3.0)) + 1       # k = 0..n/3
    mp = MP[n]
    k = np.arange(m)[:, None]
    j = np.arange(n)[None, :]
    ang = 2.0 * np.pi * k * j / n
    C = np.zeros((mp, n)); C[:m] = np.cos(ang)
    S = np.zeros((mp, n)); S[:m] = np.sin(ang)
    Lm = phi[:m, None] * phi[None, :m]
    G = 1.0 - np.sqrt(np.clip(1.0 - Lm ** 2, 0.0, None))
    w = np.full(m, 2.0); w[0] = 1.0
    Gt = np.zeros((mp, mp))
    Gt[:m, :m] = (w[:, None] * w[None, :]) * G / (n * n)
    return C, S, -Gt                      # negated: band = x^T - corr^T


def _conv_matrix(f, n):
    F = np.zeros((n, n))
    i = np.arange(n)
    for t in range(len(f)):
        F[i, (i + t - len(f) // 2) % n] += f[t]
    return F


def _dfb_gvecs(A):
    """band generator vectors g (len A) for variants (s/d) x (even/odd col)."""
    h = A // 2
    P0 = np.zeros((h, A)); P0[np.arange(h), 2 * np.arange(h)] = 1.0
    P1 = np.zeros((h, A)); P1[np.arange(h), 2 * np.arange(h) + 1] = 1.0
    F = _conv_matrix(PKVA, h)
    Md = P1 - F @ P0
    Ms = P0 + 0.5 * F @ Md
    sh = np.roll(np.eye(A), 1, axis=1)    # (sh x)[a] = x[a+1]
    return {"se": Ms[0, :].copy(), "so": (-(Ms @ sh))[0, :].copy(),
            "de": Md[0, :].copy(), "do": (-(Md @ sh))[0, :].copy()}


def _dfb_residues(A):
    nch = A // P
    seen, res = set(), []
    for o in (-1, 0, 1, 2):
        r = o % nch
        if r not in seen:
            seen.add(r); res.append(r)
    return res


def _dfb_block(g, A, r):
    """stationary lhsT block (a_local, i_local): g[(128 r + al - 2 il) mod A]"""
    al = np.arange(P)[:, None]
    il = np.arange(P)[None, :]
    return g[(P * r + al - 2 * il) % A]


VARIANTS = ("se", "so", "de", "do")


def _host_constants():
    """All constant DRAM inputs (float32), and DFB block index maps."""
    c = {}
    for n in NS:
        D = _lp_D(n)
        C, S, Gneg = _corr_CSG(n)
        c[f"Dt{n}"] = np.ascontiguousarray(D.T).astype(np.float32)       # (n, n/2)
        c[f"Ct{n}"] = np.ascontiguousarray(C.T).astype(np.float32)       # (n, mp)
        c[f"St{n}"] = np.ascontiguousarray(S.T).astype(np.float32)       # (n, mp)
        c[f"Cp{n}"] = np.ascontiguousarray(C).astype(np.float32)         # (mp, n)
        c[f"Sp{n}"] = np.ascontiguousarray(S).astype(np.float32)         # (mp, n)
        c[f"G{n}"] = np.ascontiguousarray(Gneg).astype(np.float32)       # (mp, mp)
    bidx = {}
    for A in NS:
        gv = _dfb_gvecs(A)
        blocks = []
        for key in VARIANTS:
            for r in _dfb_residues(A):
                bidx[(A, key, r)] = len(blocks)
                blocks.append(_dfb_block(gv[key], A, r).astype(np.float32))
        c[f"BLK{A}"] = np.ascontiguousarray(np.stack(blocks))  # (nblk, 128, 128)
    c["IDENT"] = np.eye(P, dtype=np.float32)
    return c, bidx


_CONSTS, _BIDX = None, None


def _get_constants():
    global _CONSTS, _BIDX
    if _CONSTS is None:
        _CONSTS, _BIDX = _host_constants()
    return _CONSTS, _BIDX


# ----------------------------------------------------------------------------
# device kernel builder
# ----------------------------------------------------------------------------
def _build_nc(n_images=2):
    import contextlib
    import concourse.bacc as bacc
    import concourse.mybir as mybir
    import concourse.tile as tile

    F32 = mybir.dt.float32
    F32R = mybir.dt.float32r
    consts, bidx = _get_constants()

    nc = bacc.Bacc("TRN2", target_bir_lowering=False, debug=False)

    Xin = nc.declare_dram_parameter("x", [n_images, 1024, 1024], F32R, isOutput=False)
    dconst = {}
    for name, arr in consts.items():
        dt = F32 if name.startswith("G") else F32R
        dconst[name] = nc.declare_dram_parameter(name, list(arr.shape), dt,
                                                 isOutput=False)
    Olp = nc.declare_dram_parameter("lp", [n_images, 128, 128], F32R, True)
    Ob1 = nc.declare_dram_parameter("b1", [n_images, 4, 128, 128], F32R, True)
    Ob2 = nc.declare_dram_parameter("b2", [n_images, 8, 256, 128], F32R, True)
    Ob3 = nc.declare_dram_parameter("b3", [n_images, 16, 256, 256], F32R, True)
    Oband = {1024: Ob3, 512: Ob2, 256: Ob1}

    copy_flip = [0]

    def copy_out(dst, src):
        """PSUM -> SBUF copy, alternating ACT/DVE."""
        copy_flip[0] ^= 1
        if copy_flip[0]:
            nc.scalar.copy(dst, src)
        else:
            nc.vector.tensor_copy(dst, src)

    with tile.TileContext(nc) as tc, contextlib.ExitStack() as ctx:
        big = ctx.enter_context(tc.tile_pool(name="big", bufs=2))
        sm = ctx.enter_context(tc.tile_pool(name="sm", bufs=2))
        lpsm = ctx.enter_context(tc.tile_pool(name="lpsm", bufs=2))
        mid = ctx.enter_context(tc.tile_pool(name="mid", bufs=2))
        w1p = ctx.enter_context(tc.tile_pool(name="w1p", bufs=2))
        ypool = ctx.enter_context(tc.tile_pool(name="ypool", bufs=4))
        cpool = ctx.enter_context(tc.tile_pool(name="cpool", bufs=2))
        gpool = ctx.enter_context(tc.tile_pool(name="gpool", bufs=1))
        dtpool = ctx.enter_context(tc.tile_pool(name="dtpool", bufs=1))
        bpool = ctx.enter_context(tc.tile_pool(name="bpool", bufs=2))
        idp = ctx.enter_context(tc.tile_pool(name="idp", bufs=1))
        accp = ctx.enter_context(tc.tile_pool(name="accp", bufs=4, space="PSUM"))
        lpp = ctx.enter_context(tc.tile_pool(name="lpp", bufs=4, space="PSUM"))

        ident = idp.tile([P, P], F32R, tag="ident")
        nc.sync.dma_start(ident[:], dconst["IDENT"][:])

        def load_const(pool, name, n_rows, width, dt, tag):
            t = pool.tile([P, n_rows // P, width], dt, tag=tag)
            nc.sync.dma_start(t[:], dconst[name].rearrange("(c p) w -> p c w", p=P))
            return t

        def load_blocks(A):
            nblk = consts[f"BLK{A}"].shape[0]
            t = bpool.tile([P, nblk, P], F32R, tag="blk")
            nc.sync.dma_start(t[:], dconst[f"BLK{A}"].rearrange("n p w -> p n w"))
            return t

        _uid = [0]

        def alloc_band(n):
            _uid[0] += 1
            if n == 1024:
                return big.tile([P, n // P, n], F32R, tag="bigb",
                                name=f"band{_uid[0]}")
            return sm.tile([P, n // P, n], F32R, tag="sm", name=f"band{_uid[0]}")

        def alloc_stack(n, chunks, width):
            _uid[0] += 1
            if n == 1024:
                return big.tile([P, chunks, width], F32R, tag="bigb",
                                name=f"stk{_uid[0]}")
            return sm.tile([P, chunks, width], F32R, tag="sm",
                           name=f"stk{_uid[0]}")

        def pyr_level(xr, n):
            """xr: [128, nch, n] f32r, natural [row, col].
            Returns (bandT [128, nch, n] f32r = band^T, lp [128, nch/2, n/2])."""
            nch = n // P
            mp = MP[n]
            mch = mp // P
            N5 = min(n, 512)
            npass = n // N5
            hw = n // 2

            # ---- LP: lp = D x D^T ----
            Dt = load_const(dtpool, f"Dt{n}", n, hw, F32R, "dt")
            lp = lpsm.tile([P, nch // 2, hw], F32R, tag="lp")
            lp_ps = [lpp.tile([P, min(hw, 512)], F32, tag="lp",
                              name=f"lp_ps{_ic}")
                     for _ic in range(nch // 2)]
            for c2 in range(nch):
                wps = accp.tile([P, hw], F32, tag="acc")
                for c1 in range(nch):
                    nc.tensor.matmul(wps[:], xr[:, c1, P*c2:P*(c2+1)],
                                     Dt[:, c1, :],
                                     start=(c1 == 0), stop=(c1 == nch - 1))
                w1 = w1p.tile([P, hw], F32R, tag="w1")
                copy_out(w1[:], wps[:])
                for ic in range(nch // 2):
                    nc.tensor.matmul(lp_ps[ic][:], w1[:, P*ic:P*(ic+1)],
                                     Dt[:, c2, :],
                                     start=(c2 == 0), stop=(c2 == nch - 1))
            for ic in range(nch // 2):
                copy_out(lp[:, ic, :], lp_ps[ic][:])

            # ---- S1: P1c = (C x)^T, P1s = (S x)^T  [col j' (p), k (f)] ----
            Ct = load_const(cpool, f"Ct{n}", n, mp, F32R, "cs")
            St = load_const(cpool, f"St{n}", n, mp, F32R, "cs")
            P1c = mid.tile([P, nch, mp], F32R, tag="mid")
            P1s = mid.tile([P, nch, mp], F32R, tag="mid")
            for c2 in range(nch):
                for dst, M in ((P1c, Ct), (P1s, St)):
                    ps = accp.tile([P, mp], F32, tag="acc")
                    for c1 in range(nch):
                        nc.tensor.matmul(ps[:], xr[:, c1, P*c2:P*(c2+1)],
                                         M[:, c1, :],
                                         start=(c1 == 0), stop=(c1 == nch - 1))
                    copy_out(dst[:, c2, :], ps[:])

            # ---- S2: Yt_i = Gneg * (xcc,xcs,xsc,xss)^T  [l (p), k (f)] ----
            G = load_const(gpool, f"G{n}", mp, mp, F32, "g")
            Ys = [ypool.tile([P, mch, mp], F32R, tag="y", name=f"Y{_i}")
                  for _i in range(4)]
            pairs = ((Ct, P1c), (St, P1c), (Ct, P1s), (St, P1s))
            for lc in range(mch):
                for yi, (Mst, Pmv) in enumerate(pairs):
                    ps = accp.tile([P, mp], F32, tag="acc")
                    for c1 in range(nch):
                        nc.tensor.matmul(ps[:], Mst[:, c1, P*lc:P*(lc+1)],
                                         Pmv[:, c1, :],
                                         start=(c1 == 0), stop=(c1 == nch - 1))
                    nc.vector.tensor_mul(Ys[yi][:, lc, :], ps[:], G[:, lc, :])

            # ---- S3: R1 = Y1t' Cp + Y2t' Sp ; R2 = Y3t' Cp + Y4t' Sp ----
            Cp = load_const(cpool, f"Cp{n}", mp, n, F32R, "cs")
            Sp = load_const(cpool, f"Sp{n}", mp, n, F32R, "cs")
            R1 = mid.tile([P, mch, n], F32R, tag="mid")
            R2 = mid.tile([P, mch, n], F32R, tag="mid")
            for kc in range(mch):
                for bp in range(npass):
                    bsl = slice(N5 * bp, N5 * (bp + 1))
                    for R, Ya, Yb in ((R1, Ys[0], Ys[1]), (R2, Ys[2], Ys[3])):
                        ps = accp.tile([P, N5], F32, tag="acc")
                        for c1 in range(mch):
                            nc.tensor.matmul(ps[:], Ya[:, c1, P*kc:P*(kc+1)],
                                             Cp[:, c1, bsl],
                                             start=(c1 == 0), stop=False)
                            nc.tensor.matmul(ps[:], Yb[:, c1, P*kc:P*(kc+1)],
                                             Sp[:, c1, bsl],
                                             start=False, stop=(c1 == mch - 1))
                        copy_out(R[:, kc, bsl], ps[:])

            # ---- S4: band^T = x^T (PE transposes) + (-corr^T) ----
            bandT = alloc_band(n)
            for bc in range(nch):
                for ap_ in range(npass):
                    asl = slice(N5 * ap_, N5 * (ap_ + 1))
                    ps = accp.tile([P, N5], F32, tag="acc")
                    for c1 in range(mch):
                        nc.tensor.matmul(ps[:], R1[:, c1, P*bc:P*(bc+1)],
                                         Cp[:, c1, asl],
                                         start=(c1 == 0), stop=False,
                                         skip_group_check=True)
                        nc.tensor.matmul(ps[:], R2[:, c1, P*bc:P*(bc+1)],
                                         Sp[:, c1, asl],
                                         start=False, stop=False,
                                         skip_group_check=True)
                    ntp = N5 // P
                    for t in range(ntp):
                        ac = ap_ * ntp + t
                        nc.tensor.matmul(
                            ps[:, P*t:P*(t+1)].bitcast(F32R),
                            xr[:, ac, P*bc:P*(bc+1)], ident[:],
                            is_transpose=True, start=False,
                            stop=(t == ntp - 1), skip_group_check=True)
                    copy_out(bandT[:, bc, asl], ps[:])
            return bandT, lp

        def transpose_stack(stack, nsub, A, B, n):
            """stack [128, nsub*(A/128), B] -> out [128, nsub*(B/128), A]."""
            ach, bch = A // P, B // P
            out = alloc_stack(n, nsub * bch, A)
            for s in range(nsub):
                for bc in range(bch):
                    for ap_ in range((A + 511) // 512):
                        Na = min(512, A - 512 * ap_)
                        nt = Na // P
                        ps = accp.tile([P, 512], F32, tag="acc")
                        for t in range(nt):
                            nc.tensor.matmul(
                                ps[:, P*t:P*(t+1)].bitcast(F32R),
                                stack[:, s * ach + ap_ * 4 + t, P*bc:P*(bc+1)],
                                ident[:], is_transpose=True,
                                start=(t == 0), stop=(t == nt - 1),
                                skip_group_check=True)
                        copy_out(out[:, s * bch + bc, 512*ap_:512*ap_ + Na],
                                 ps[:, :Na])
            return out

        def dfb_level(stack, nsub, A, B, n, blk):
            """One fan_split level in P-form: stack [128, nsub*(A/128), B]
            -> out [128, (2 nsub)*(A/256), B], s subbands then d subbands."""
            ach = A // P
            och = ach // 2
            out = alloc_stack(n, 2 * nsub * och, B)
            res = _dfb_residues(A)
            nres = len(res)
            for s in range(nsub):
                for ki, kind in enumerate("sd"):
                    for I in range(och):
                        for bp in range((B + 511) // 512):
                            Bn = min(512, B - 512 * bp)
                            Bh = Bn // 2
                            ps = accp.tile([P, 512], F32, tag="acc")
                            for pi, par in enumerate("eo"):
                                for ri, r in enumerate(res):
                                    J = (2 * I + r) % ach
                                    bt = blk[:, bidx[(A, kind + par, r)], :]
                                    mv = stack[:, s * ach + J,
                                               512 * bp + pi: 512 * bp + Bn: 2]
                                    # f32r psum out must be contiguous: parity
                                    # halves land side by side, interleaved on
                                    # the copy out below.
                                    nc.tensor.matmul(
                                        ps[:, pi * Bh:(pi + 1) * Bh], bt, mv,
                                        start=(ri == 0), stop=(ri == nres - 1),
                                        skip_group_check=True)
                            dst = out[:, (ki * nsub + s) * och + I, :]
                            for pi in range(2):
                                copy_out(dst[:, 512 * bp + pi: 512 * bp + Bn: 2],
                                         ps[:, pi * Bh:(pi + 1) * Bh])
            return out

        def dfb(img, bandT, n):
            """bandT [128, nch, n] = band^T ([col (p), row (f)])."""
            nlev = NLEV_DFB[n]
            stack, nsub = bandT, 1
            A, B = n, n
            blk, blk_A = None, None
            for lev in range(nlev):
                if blk_A != A:
                    blk, blk_A = load_blocks(A), A
                stack = dfb_level(stack, nsub, A, B, n, blk)
                nsub *= 2
                A //= 2
                if lev < nlev - 1:
                    stack = transpose_stack(stack, nsub, A, B, n)
                    A, B = B, A
            if nlev % 2 == 1:  # ends transposed -> fix orientation
                stack = transpose_stack(stack, nsub, A, B, n)
                A, B = B, A
            och = A // P
            for s in range(nsub):
                nc.sync.dma_start(
                    Oband[n][img, s].rearrange("(c p) w -> p c w", p=P),
                    stack[:, s * och:(s + 1) * och, :])

        for img in range(n_images):
            xr = big.tile([P, 8, 1024], F32R, tag="bigb")
            nc.sync.dma_start(xr[:], Xin[img].rearrange("(c p) w -> p c w", p=P))
            cur = xr
            for n in NS:
                bandT, lp = pyr_level(cur, n)
                dfb(img, bandT, n)
                cur = lp
            nc.sync.dma_start(Olp[img].rearrange("(c p) w -> p c w", p=P),
                              cur[:, 0:1, :])

    nc.finalize()
    return nc


# ----------------------------------------------------------------------------
# public entry
# ----------------------------------------------------------------------------
def run_cores(x, trace=False):
    """x: (16, 1, 1024, 1024) fp32 -> (results list, BassKernelResults)."""
    from concourse.bass_utils import run_bass_kernel_spmd

    x = np.asarray(x)
    B = x.shape[0]
    n_cores = int(os.environ.get("KERNEL_CORES", "8"))
    per = B // n_cores
    consts, _ = _get_constants()

    nc = _build_nc(n_images=per)
    in_maps = []
    for c in range(n_cores):
        m = {"x": np.ascontiguousarray(x[c * per:(c + 1) * per, 0])}
        m.update(consts)
        in_maps.append(m)
    res = run_bass_kernel_spmd(nc, in_maps, core_ids=list(range(n_cores)),
                               trace=trace)
    return res


def kernel(x):
    """x: (16, 1, 1024, 1024) float32. Returns the contourlet tuple:
    ((16,1,128,128), (4,16,1,128,128), (8,16,1,256,128), (16,16,1,256,256))."""
    res = run_cores(x)
    lp = np.concatenate([r["lp"] for r in res.results], 0)[:, None]
    b1 = np.concatenate([r["b1"] for r in res.results], 0)
    b2 = np.concatenate([r["b2"] for r in res.results], 0)
    b3 = np.concatenate([r["b3"] for r in res.results], 0)
    b1 = np.moveaxis(b1, 0, 1)[:, :, None]
    b2 = np.moveaxis(b2, 0, 1)[:, :, None]
    b3 = np.moveaxis(b3, 0, 1)[:, :, None]
    return (lp, b1, b2, b3)


if __name__ == "__main__":
    xs = np.random.default_rng(0).standard_normal((16, 1, 1024, 1024)).astype(np.float32)
    outs = kernel(xs)
    print([o.shape for o in outs])
